# revision 51
# baseline (speedup 1.0000x reference)
"""Trainium2 Bass kernel for per-component tiny-MLP stack (CFCN constructor).

Computation (reference):
    h1 = relu(x[:, :, None] * W1 + b1)            # [B, D, H]
    h2 = relu(einsum('bdh,dhk->bdk', h1, W2) + b2)  # [B, D, H]
    out = einsum('bdh,dh->bd', h2, W3) + b3        # [B, D]

B=16384, D=64, H=128. Sharded over 8 NeuronCores by component: 8 components
per core, full batch per core (fully independent per-component MLPs — no
collectives needed).

Per-core dataflow (everything transposed: H on partitions, batch on free):
  L1: PE outer-product matmuls, K=2 (x row + ones row) so the bias rides in
      the contraction; 2-way row-strip packing (tile_position) so two
      components' L1 matmuls run concurrently.
  h1 = relu(z1): PSUM->SBUF eviction on ScalarE/VectorE (alternating).
  L2: K=128 fp32r matmul with W2_d stationary.
  h2 = relu(z2 + b2): eviction with per-partition bias.
  L3: per batch window, 8 accumulating M=8 fp32r matmuls (stationary =
      [128, 8] with only column d nonzero = W3_d) sum into one PSUM bank with
      the 8 components' outputs on contiguous partitions 0..7, evicted as a
      legal [8, 512] engine copy and DMA'd out.
  b3 and final transpose applied on host.
"""

import sys

if "/opt/trn_rl_repo" not in sys.path:
    sys.path.insert(0, "/opt/trn_rl_repo")

import numpy as np

B, D, H = 16384, 64, 128
NCORES = 8
DPC = D // NCORES  # components per core = 8
P = 128
W_ = 512           # batch window per matmul (fp32 moving-operand max)
BT = 2048          # batch chunk per xones tile
NBT = B // BT      # 8
NG = DPC // 2      # component pairs per core = 4

_CACHE = {}


def _build_program():
    from collections import deque
    from contextlib import ExitStack

    from concourse import bass, mybir
    from concourse import tile
    from concourse.tile_rust import add_dep_helper

    f32 = mybir.dt.float32
    f32r = mybir.dt.float32r
    Relu = mybir.ActivationFunctionType.Relu
    Copy = mybir.ActivationFunctionType.Copy
    Alu = mybir.AluOpType

    nc = bass.Bass("TRN2", target_bir_lowering=False, debug=False)

    # DRAM I/O (per-core data supplied via in_maps)
    xa = nc.dram_tensor("xa", [2 * DPC, B], f32r, kind="ExternalInput")
    wpk = nc.dram_tensor("wpk", [P, NG * H], f32r, kind="ExternalInput")
    w2 = nc.dram_tensor("w2", [H, DPC * H], f32r, kind="ExternalInput")
    b2t = nc.dram_tensor("b2t", [H, DPC], f32, kind="ExternalInput")
    # W3 embeddings: w3e[:, 8*d : 8*d+8] is [H, 8] with only column d nonzero
    w3e = nc.dram_tensor("w3e", [H, DPC * DPC], f32r, kind="ExternalInput")
    # [bt, w, d, 512] so the per-(bt,w) staging tile DMAs out with a natural AP
    o = nc.dram_tensor("o", [NBT, BT // W_, DPC, W_], f32, kind="ExternalOutput")

    ecnt = [0]

    with tile.TileContext(nc) as tc, ExitStack() as ctx:
        wts = ctx.enter_context(tc.tile_pool(name="wts", bufs=1))
        xo_pool = ctx.enter_context(tc.tile_pool(name="xo", bufs=3))
        z1_pool = ctx.enter_context(tc.tile_pool(name="z1", bufs=2, space="PSUM"))
        h1_pool = ctx.enter_context(tc.tile_pool(name="h1", bufs=4))
        z2_pool = ctx.enter_context(tc.tile_pool(name="z2", bufs=2, space="PSUM"))
        h2_pool = ctx.enter_context(tc.tile_pool(name="h2", bufs=18))
        ost_pool = ctx.enter_context(tc.tile_pool(name="ost", bufs=3))

        wpk_sb = wts.tile([P, NG * H], f32r)
        wd_wpk = nc.sync.dma_start(wpk_sb[:], wpk[:, :])
        w2_sb = wts.tile([H, DPC * H], f32r)
        wd_w2 = nc.sync.dma_start(w2_sb[:], w2[:, :])
        b2_sb = wts.tile([H, DPC], f32)
        wd_b2 = nc.sync.dma_start(b2_sb[:], b2t[:, :])
        w3_sb = wts.tile([H, DPC * DPC], f32r)
        wd_w3 = nc.sync.dma_start(w3_sb[:], w3e[:, :])

        def evict(dst, src, bias_col, use_act=None):
            # dst = relu(src + bias); alternate ScalarE (5/9) and VectorE (4/9)
            # to balance the two engines' eviction throughput.
            if use_act is None:
                use_act = (ecnt[0] * 5) % 9 < 5
            ecnt[0] += 1
            if use_act:
                if bias_col is None:
                    return nc.scalar.activation(dst[:], src[:], Relu)
                return nc.scalar.activation(dst[:], src[:], Relu, bias=bias_col)
            if bias_col is None:
                return nc.vector.tensor_scalar(dst[:], src[:], 0.0, None, Alu.max)
            return nc.vector.tensor_scalar(
                dst[:], src[:], bias_col, 0.0, Alu.add, Alu.max
            )

        def evict_copy(dst, src, use_act=None):
            if use_act is None:
                use_act = (ecnt[0] * 5) % 9 < 5
            ecnt[0] += 1
            if use_act:
                return nc.scalar.activation(dst[:], src[:], Copy)
            return nc.vector.tensor_copy(dst[:], src[:])

        # Self-loading fp32r matmuls only have ONE sync-wait slot in walrus
        # codegen. Absorb extra cross-engine waits into PE nops placed just
        # before each matmul group: the nop waits, the PE FIFO order covers
        # the matmul, and Tile's vector clock elides the duplicate wait.
        def pe_absorb(producers):
            # returns nops (created BEFORE the matmuls they shield) — caller
            # must order the first matmul after them via pe_order()
            nops = []
            for p in producers:
                if p is None:
                    continue
                n = nc.tensor.nop()
                add_dep_helper(n.ins, p.ins, True, "wait-carrier")
                nops.append(n)
            return nops

        def pe_order(first_mm, nops):
            for n in nops:
                add_dep_helper(first_mm.ins, n.ins, False, "carrier-order")

        # slot-freeing instruction trackers (bufs=2 pools)
        zslot = deque(maxlen=2)   # tag "z1" (z1 tiles + op tiles)
        z2slot = deque(maxlen=2)  # z2 tiles

        wdmas = [wd_wpk, wd_w2, wd_b2, wd_w3]
        for bt in range(NBT):
            h2s = {}
            for g in range(NG):
                xo = xo_pool.tile([P, BT], f32r)
                sl = slice(bt * BT, (bt + 1) * BT)
                xdA = nc.sync.dma_start(xo[0:2, :], xa[4 * g : 4 * g + 2, sl])
                xdB = nc.sync.dma_start(xo[32:34, :], xa[4 * g + 2 : 4 * g + 4, sl])
                xo_deps = [xdA, xdB] + wdmas
                wdmas = []

                for wp in range(2):
                    unit_act = (2 * g + wp) % 2 == 0
                    h1s = []
                    for q in range(2):
                        w = 2 * wp + q
                        deps = list(xo_deps)
                        xo_deps = []
                        if len(zslot) == zslot.maxlen:
                            deps.append(zslot[0])
                        nops = pe_absorb(deps)
                        z1 = z1_pool.tile([P, 2 * W_], f32)
                        mm0 = None
                        for s in range(2):
                            mm = nc.tensor.matmul(
                                z1[:, s * W_ : (s + 1) * W_],
                                lhsT=wpk_sb[32 * s : 32 * s + 2, g * H : (g + 1) * H],
                                rhs=xo[32 * s : 32 * s + 2, w * W_ : (w + 1) * W_],
                                start=True,
                                stop=True,
                                tile_position=(32 * s, 0),
                            )
                            mm0 = mm0 or mm
                        pe_order(mm0, nops)
                        h1 = h1_pool.tile([P, 2 * W_], f32r)
                        zslot.append(evict(h1, z1, None, use_act=unit_act))
                        h1s.append(h1)
                    for s in range(2):
                        di = 2 * g + s
                        nops = (
                            pe_absorb([z2slot[0]])
                            if len(z2slot) == z2slot.maxlen
                            else []
                        )
                        z2 = z2_pool.tile([P, 2 * W_], f32)
                        mm0 = None
                        for q in range(2):
                            mm = nc.tensor.matmul(
                                z2[:, q * W_ : (q + 1) * W_],
                                lhsT=w2_sb[:, di * H : (di + 1) * H],
                                rhs=h1s[q][:, s * W_ : (s + 1) * W_],
                                start=True,
                                stop=True,
                            )
                            mm0 = mm0 or mm
                        pe_order(mm0, nops)
                        h2 = h2_pool.tile([P, 2 * W_], f32r)
                        z2slot.append(
                            evict(h2, z2, b2_sb[:, di : di + 1], use_act=unit_act)
                        )
                        h2s[(di, wp)] = h2

            # L3 for the whole bt chunk: for each 512-window accumulate all 8
            # components into PSUM partitions 0..7 (W3-embedding stationaries).
            for w in range(4):
                wp, q = w // 2, w % 2
                # share the z1 pool's PSUM slots — op is tiny and the L3
                # phase interleaves with the next bt's L1 fills
                nops = pe_absorb([zslot[0]]) if len(zslot) == zslot.maxlen else []
                op = z1_pool.tile([DPC, W_], f32, tag="z1")
                mm0 = None
                for di in range(DPC):
                    mm = nc.tensor.matmul(
                        op[:, :],
                        lhsT=w3_sb[:, DPC * di : DPC * (di + 1)],
                        rhs=h2s[(di, wp)][:, q * W_ : (q + 1) * W_],
                        start=(di == 0),
                        stop=(di == DPC - 1),
                    )
                    mm0 = mm0 or mm
                pe_order(mm0, nops)
                ost = ost_pool.tile([DPC, W_], f32)
                zslot.append(evict_copy(ost, op[:]))
                nc.sync.dma_start(o[bt, w], ost[:])

    return nc


def _build_program_raw():
    """Raw-bass build: manual engine streams + counting semaphores.

    Self-loading fp32r matmuls only support ONE sync-wait in walrus codegen,
    so all multi-proc waits are standalone wait_ge instructions placed by
    hand. PSUM: 4 ping-pong pairs of [128, 1024] (z1 A/B, z2 A/B); the L3
    `op` accumulators time-share the z1 banks at each bt boundary.
    """
    from concourse import bass, mybir

    f32 = mybir.dt.float32
    f32r = mybir.dt.float32r
    Relu = mybir.ActivationFunctionType.Relu
    Copy = mybir.ActivationFunctionType.Copy
    Alu = mybir.AluOpType

    nc = bass.Bass("TRN2", target_bir_lowering=False, debug=False)

    xa = nc.dram_tensor("xa", [2 * DPC, B], f32r, kind="ExternalInput")
    wpk = nc.dram_tensor("wpk", [P, NG * H], f32r, kind="ExternalInput")
    w2 = nc.dram_tensor("w2", [H, DPC * H], f32r, kind="ExternalInput")
    b2t = nc.dram_tensor("b2t", [H, DPC], f32, kind="ExternalInput")
    w3e = nc.dram_tensor("w3e", [H, DPC * DPC], f32r, kind="ExternalInput")
    o = nc.dram_tensor("o", [NBT, BT // W_, DPC, W_], f32, kind="ExternalOutput")

    # SBUF
    wpk_sb = nc.alloc_sbuf_tensor("wpk_sb", [P, NG * H], f32r)
    w2_sb = nc.alloc_sbuf_tensor("w2_sb", [H, DPC * H], f32r)
    b2_sb = nc.alloc_sbuf_tensor("b2_sb", [H, DPC], f32)
    w3_sb = nc.alloc_sbuf_tensor("w3_sb", [H, DPC * DPC], f32r)
    xo = [nc.alloc_sbuf_tensor(f"xo{i}", [P, BT], f32r) for i in range(2)]
    h1b = [nc.alloc_sbuf_tensor(f"h1b{i}", [P, 2 * W_], f32r) for i in range(4)]
    h2b = [
        [nc.alloc_sbuf_tensor(f"h2b{wp}_{d}", [P, 2 * W_], f32r) for d in range(DPC)]
        for wp in range(2)
    ]
    ost = [nc.alloc_sbuf_tensor(f"ost{i}", [DPC, W_], f32) for i in range(4)]

    # PSUM: zb0/zb1 = z1 ping-pong (+ L3 op at bt ends), zb2/zb3 = z2 ping-pong
    zb = [nc.alloc_psum_tensor(f"zb{i}", [P, 2 * W_], f32) for i in range(4)]

    # semaphores
    s_wdma = nc.alloc_semaphore("s_wdma")
    s_x = [nc.alloc_semaphore(f"s_x{i}") for i in range(2)]
    s_od = [nc.alloc_semaphore(f"s_od{i}") for i in range(4)]
    s_z1 = nc.alloc_semaphore("s_z1")
    s_z2 = nc.alloc_semaphore("s_z2")
    s_op = nc.alloc_semaphore("s_op")
    s_h1 = {"a": nc.alloc_semaphore("s_h1a"), "d": nc.alloc_semaphore("s_h1d")}
    s_h2 = {"a": nc.alloc_semaphore("s_h2a"), "d": nc.alloc_semaphore("s_h2d")}
    s_oc = {"a": nc.alloc_semaphore("s_oca"), "d": nc.alloc_semaphore("s_ocd")}

    NU = NBT * NG * 2  # 64 units; unit u = (bt, g, wp)

    # Chain-to-engine mapping: fill index j (j = 2u + q for z1, 2u + s for
    # z2) has parity-based ownership: even -> ACT ("a"), odd -> DVE ("d").
    # Each engine serves its chains strictly in order, so the engine's
    # counting semaphore value for evict j is simply j//2 + 1.
    def ev_eng(j):
        return "a" if j % 2 == 0 else "d"

    with nc.Block() as block:

        @block.sync
        def _(sp):
            sp.dma_start(wpk_sb[:, :], wpk[:, :]).then_inc(s_wdma, 16)
            sp.dma_start(w2_sb[:, :], w2[:, :]).then_inc(s_wdma, 16)
            sp.dma_start(b2_sb[:, :], b2t[:, :]).then_inc(s_wdma, 16)
            sp.dma_start(w3_sb[:, :], w3e[:, :]).then_inc(s_wdma, 16)
            for bt in range(NBT + 1):
                if bt < NBT:
                    for g in range(NG):
                        idx = bt * NG + g
                        xi = idx % 2
                        if idx >= 2:
                            # xo[xi] last read by L1 fills of (bt,g)-2:
                            # those are z1 fills 4*(idx-2)+1 .. 4*(idx-1)
                            sp.wait_ge(s_z1, 4 * (idx - 1))
                        sl = slice(bt * BT, (bt + 1) * BT)
                        sp.dma_start(
                            xo[xi][0:2, :], xa[4 * g : 4 * g + 2, sl]
                        ).then_inc(s_x[xi], 16)
                        sp.dma_start(
                            xo[xi][32:34, :], xa[4 * g + 2 : 4 * g + 4, sl]
                        ).then_inc(s_x[xi], 16)
                # out DMAs of the previous bt (out-copies all run on ACT)
                if bt >= 1:
                    for w in range(4):
                        k = (bt - 1) * 4 + w
                        sp.wait_ge(s_oc["a"], k + 1)
                        sp.dma_start(o[bt - 1, w], ost[k % 4][:, :]).then_inc(
                            s_od[k % 4], 16
                        )

        UPB = NG * 2  # units per bt

        ENG_OF = ("a", "d")

        def pe_z1_fill(pe, u, q):
            # one z1 fill (unit u, window-pair column q) into zb[q]
            bt, r = divmod(u, UPB)
            g, wp = r // 2, r % 2
            idx = bt * NG + g
            xi = idx % 2
            if wp == 0 and q == 0:
                pe.wait_ge(s_x[xi], 32 * (idx // 2 + 1))
            mm = None
            for s in range(2):
                mm = pe.matmul(
                    zb[q][:, s * W_ : (s + 1) * W_],
                    lhsT=wpk_sb[32 * s : 32 * s + 2, g * H : (g + 1) * H],
                    rhs=xo[xi][32 * s : 32 * s + 2, w_slice(wp, q)],
                    start=True,
                    stop=True,
                    tile_position=(32 * s, 0),
                )
                if u >= 1:
                    # WAR: the s-slice of fill 2(u-1)+q was evicted by
                    # engine s's half-evict
                    mm._wait_ge(s_h1[ENG_OF[s]], 2 * (u - 1) + q + 1)
            mm.then_inc(s_z1, 1)

        def pe_z2_fill(pe, v, s):
            bt, r = divmod(v, UPB)
            g, wp = r // 2, r % 2
            di = 2 * g + s
            mm = None
            for q in range(2):
                mm = pe.matmul(
                    zb[2 + s][:, q * W_ : (q + 1) * W_],
                    lhsT=w2_sb[:, di * H : (di + 1) * H],
                    rhs=h1b[(v % 2) * 2 + q][:, s * W_ : (s + 1) * W_],
                    start=True,
                    stop=True,
                )
                # ready: h1b fill 2v+q's s-half (engine s) evicted
                mm._wait_ge(s_h1[ENG_OF[s]], 2 * v + q + 1)
            mm.then_inc(s_z2, 1)

        def pe_l3_phase(pe, bt):
            # op(w) lives in zb[2 + w % 2][0:8, (w // 2)*512 :] — the z2
            # banks, so the next bt's z1 chains flow undisturbed.
            pe.wait_ge(s_h2["a"], 2 * UPB * (bt + 1))
            pe.wait_ge(s_h2["d"], 2 * UPB * (bt + 1))
            for w in range(4):
                wp, q = w // 2, w % 2
                opv = zb[2 + w % 2][0:DPC, (w // 2) * W_ : (w // 2 + 1) * W_]
                mm = None
                for di in range(DPC):
                    mm = pe.matmul(
                        opv,
                        lhsT=w3_sb[:, DPC * di : DPC * (di + 1)],
                        rhs=h2b[wp][di][:, q * W_ : (q + 1) * W_],
                        start=(di == 0),
                        stop=(di == DPC - 1),
                    )
                mm.then_inc(s_op, 1)

        @block.tensor
        def _(pe):
            pe.wait_ge(s_wdma, 64)
            for t in range(NU + 1):
                # slot t (spread order): zb0 fill early, z2 fills mid,
                # L3 phase at bt boundaries, zb1 fill late.
                if t < NU:
                    pe_z1_fill(pe, t, 0)
                if t >= 1:
                    v = t - 1
                    if v >= 1:
                        # zb2/zb3 WAR: both half-evicts of fills 2(v-1)+s
                        pe.wait_ge(s_h2["a"], 2 * v)
                        pe.wait_ge(s_h2["d"], 2 * v)
                    if v % UPB == 0 and v // UPB > 0:
                        # zb2/zb3 op regions read by out-copies of prev bt
                        pe.wait_ge(s_oc["a"], 4 * (v // UPB))
                    pe_z2_fill(pe, v, 0)
                    pe_z2_fill(pe, v, 1)
                    if t % UPB == 0:
                        pe_l3_phase(pe, t // UPB - 1)
                if t < NU:
                    pe_z1_fill(pe, t, 1)

        # Each eviction is split in half along the free dim: ACT does
        # [:, 0:512], DVE does [:, 512:1024], concurrently. Engine sem
        # count for fill j is then j+1 on BOTH s_h1a/s_h1d (resp. h2).
        def ev_h1_half(eng, mine, u, q):
            par = 0 if mine == "a" else 1
            j = 2 * u + q
            hs = slice(par * W_, (par + 1) * W_)
            if u >= 2:
                # h1b[(u%2)*2+q] last read by L2 fills of unit u-2
                eng.wait_ge(s_z2, 2 * (u - 2) + 2)
            dst = h1b[(u % 2) * 2 + q][:, hs]
            ins = (
                eng.activation(dst, zb[q][:, hs], Relu)
                if mine == "a"
                else eng.tensor_scalar(dst, zb[q][:, hs], 0.0, None, Alu.max)
            )
            ins._wait_ge(s_z1, j + 1)
            ins.then_inc(s_h1[mine], 1)

        def ev_h2_half(eng, mine, v, s):
            par = 0 if mine == "a" else 1
            bt, r = divmod(v, NG * 2)
            g, wp = r // 2, r % 2
            j = 2 * v + s
            di = 2 * g + s
            hs = slice(par * W_, (par + 1) * W_)
            if bt > 0 and r == 0 and s == 0:
                eng.wait_ge(s_op, 4 * bt)  # h2b reuse WAR
            dst = h2b[wp][di][:, hs]
            ins = (
                eng.activation(dst, zb[2 + s][:, hs], Relu, bias=b2_sb[:, di : di + 1])
                if mine == "a"
                else eng.tensor_scalar(
                    dst,
                    zb[2 + s][:, hs],
                    b2_sb[:, di : di + 1],
                    0.0,
                    Alu.add,
                    Alu.max,
                )
            )
            ins._wait_ge(s_z2, j + 1)
            ins.then_inc(s_h2[mine], 1)

        def evict_stream(eng, mine):
            eng.wait_ge(s_wdma, 64)
            for t in range(NU + 1):
                if t < NU:
                    ev_h1_half(eng, mine, t, 0)
                if t >= 1:
                    ev_h2_half(eng, mine, t - 1, 0)
                    ev_h2_half(eng, mine, t - 1, 1)
                    if t % (NG * 2) == 0 and mine == "a":
                        bt = t // (NG * 2) - 1
                        for w in range(4):
                            k = bt * 4 + w
                            if k >= 4:
                                eng.wait_ge(s_od[k % 4], 16 * (k // 4))
                            opv = zb[2 + w % 2][
                                0:DPC, (w // 2) * W_ : (w // 2 + 1) * W_
                            ]
                            ins = eng.activation(ost[k % 4][:, :], opv, Copy)
                            ins._wait_ge(s_op, k + 1)
                            ins.then_inc(s_oc[mine], 1)
                if t < NU:
                    ev_h1_half(eng, mine, t, 1)

        @block.scalar
        def _(act):
            evict_stream(act, "a")

        @block.vector
        def _(dve):
            evict_stream(dve, "d")

    return nc


def w_slice(wp, q):
    w = 2 * wp + q
    return slice(w * W_, (w + 1) * W_)


def _prep_inputs(x, W1, b1, W2, b2, W3):
    """Build the per-core input maps (host-side shard + layout transforms)."""
    in_maps = []
    for c in range(NCORES):
        dlo = c * DPC
        dc = slice(dlo, dlo + DPC)

        xa = np.empty((2 * DPC, B), np.float32)
        xa[0::2] = x.T[dc]
        xa[1::2] = 1.0

        wpk = np.zeros((P, NG * H), np.float32)
        for g in range(NG):
            for s in range(2):
                d = dlo + 2 * g + s
                wpk[32 * s, g * H : (g + 1) * H] = W1[d]
                wpk[32 * s + 1, g * H : (g + 1) * H] = b1[d]

        w2c = np.ascontiguousarray(
            W2[dc].transpose(1, 0, 2).reshape(H, DPC * H)
        ).astype(np.float32)

        w3e = np.zeros((H, DPC * DPC), np.float32)
        for i in range(DPC):
            w3e[:, DPC * i + i] = W3[dlo + i]

        in_maps.append(
            {
                "xa": xa,
                "wpk": wpk,
                "w2": w2c,
                "b2t": np.ascontiguousarray(b2[dc].T).astype(np.float32),
                "w3e": w3e,
            }
        )
    return in_maps


def run_on_hw(in_maps, trace=False):
    from concourse.bass_utils import run_bass_kernel_spmd

    if "nc" not in _CACHE:
        _CACHE["nc"] = _build_program_raw()
    nc = _CACHE["nc"]
    res = run_bass_kernel_spmd(
        nc, in_maps, list(range(NCORES)), trace=trace
    )
    return res


def _gather(results, b3):
    out = np.empty((B, D), np.float32)
    for c in range(NCORES):
        dlo = c * DPC
        # o is [bt, w, d, 512] -> [d, B]
        oc = results[c]["o"].transpose(2, 0, 1, 3).reshape(DPC, B)
        out[:, dlo : dlo + DPC] = (oc + b3[dlo : dlo + DPC][:, None]).T
    return out


def kernel(x, W1, b1, W2, b2, W3, b3):
    x = np.asarray(x, np.float32)
    W1 = np.asarray(W1, np.float32)
    b1 = np.asarray(b1, np.float32)
    W2 = np.asarray(W2, np.float32)
    b2 = np.asarray(b2, np.float32)
    W3 = np.asarray(W3, np.float32)
    b3 = np.asarray(b3, np.float32)

    in_maps = _prep_inputs(x, W1, b1, W2, b2, W3)
    res = run_on_hw(in_maps)
    return _gather(res.results, b3)


# revision 63
# speedup vs baseline: 1.0123x; 1.0123x over previous
"""Trainium2 Bass kernel for per-component tiny-MLP stack (CFCN constructor).

Computation (reference):
    h1 = relu(x[:, :, None] * W1 + b1)            # [B, D, H]
    h2 = relu(einsum('bdh,dhk->bdk', h1, W2) + b2)  # [B, D, H]
    out = einsum('bdh,dh->bd', h2, W3) + b3        # [B, D]

B=16384, D=64, H=128. Sharded over 8 NeuronCores by component: 8 components
per core, full batch per core (fully independent per-component MLPs — no
collectives needed).

Per-core dataflow (everything transposed: H on partitions, batch on free):
  L1: PE outer-product matmuls, K=2 (x row + ones row) so the bias rides in
      the contraction; 2-way row-strip packing (tile_position) so two
      components' L1 matmuls run concurrently.
  h1 = relu(z1): PSUM->SBUF eviction on ScalarE/VectorE (alternating).
  L2: K=128 fp32r matmul with W2_d stationary.
  h2 = relu(z2 + b2): eviction with per-partition bias.
  L3: per batch window, 8 accumulating M=8 fp32r matmuls (stationary =
      [128, 8] with only column d nonzero = W3_d) sum into one PSUM bank with
      the 8 components' outputs on contiguous partitions 0..7, evicted as a
      legal [8, 512] engine copy and DMA'd out.
  b3 and final transpose applied on host.
"""

import sys

if "/opt/trn_rl_repo" not in sys.path:
    sys.path.insert(0, "/opt/trn_rl_repo")

import numpy as np

B, D, H = 16384, 64, 128
NCORES = 8
DPC = D // NCORES  # components per core = 8
P = 128
W_ = 512           # batch window per matmul (fp32 moving-operand max)
BT = 2048          # batch chunk per xones tile
NBT = B // BT      # 8
NG = DPC // 2      # component pairs per core = 4

_CACHE = {}


def _build_program():
    from collections import deque
    from contextlib import ExitStack

    from concourse import bass, mybir
    from concourse import tile
    from concourse.tile_rust import add_dep_helper

    f32 = mybir.dt.float32
    f32r = mybir.dt.float32r
    Relu = mybir.ActivationFunctionType.Relu
    Copy = mybir.ActivationFunctionType.Copy
    Alu = mybir.AluOpType

    nc = bass.Bass("TRN2", target_bir_lowering=False, debug=False)

    # DRAM I/O (per-core data supplied via in_maps)
    xa = nc.dram_tensor("xa", [2 * DPC, B], f32r, kind="ExternalInput")
    wpk = nc.dram_tensor("wpk", [P, NG * H], f32r, kind="ExternalInput")
    w2 = nc.dram_tensor("w2", [H, DPC * H], f32r, kind="ExternalInput")
    b2t = nc.dram_tensor("b2t", [H, DPC], f32, kind="ExternalInput")
    # W3 embeddings: w3e[:, 8*d : 8*d+8] is [H, 8] with only column d nonzero
    w3e = nc.dram_tensor("w3e", [H, DPC * DPC], f32r, kind="ExternalInput")
    # [bt, w, d, 512] so the per-(bt,w) staging tile DMAs out with a natural AP
    o = nc.dram_tensor("o", [NBT, BT // W_, DPC, W_], f32, kind="ExternalOutput")

    ecnt = [0]

    with tile.TileContext(nc) as tc, ExitStack() as ctx:
        wts = ctx.enter_context(tc.tile_pool(name="wts", bufs=1))
        xo_pool = ctx.enter_context(tc.tile_pool(name="xo", bufs=3))
        z1_pool = ctx.enter_context(tc.tile_pool(name="z1", bufs=2, space="PSUM"))
        h1_pool = ctx.enter_context(tc.tile_pool(name="h1", bufs=4))
        z2_pool = ctx.enter_context(tc.tile_pool(name="z2", bufs=2, space="PSUM"))
        h2_pool = ctx.enter_context(tc.tile_pool(name="h2", bufs=18))
        ost_pool = ctx.enter_context(tc.tile_pool(name="ost", bufs=3))

        wpk_sb = wts.tile([P, NG * H], f32r)
        wd_wpk = nc.sync.dma_start(wpk_sb[:], wpk[:, :])
        w2_sb = wts.tile([H, DPC * H], f32r)
        wd_w2 = nc.sync.dma_start(w2_sb[:], w2[:, :])
        b2_sb = wts.tile([H, DPC], f32)
        wd_b2 = nc.sync.dma_start(b2_sb[:], b2t[:, :])
        w3_sb = wts.tile([H, DPC * DPC], f32r)
        wd_w3 = nc.sync.dma_start(w3_sb[:], w3e[:, :])

        def evict(dst, src, bias_col, use_act=None):
            # dst = relu(src + bias); alternate ScalarE (5/9) and VectorE (4/9)
            # to balance the two engines' eviction throughput.
            if use_act is None:
                use_act = (ecnt[0] * 5) % 9 < 5
            ecnt[0] += 1
            if use_act:
                if bias_col is None:
                    return nc.scalar.activation(dst[:], src[:], Relu)
                return nc.scalar.activation(dst[:], src[:], Relu, bias=bias_col)
            if bias_col is None:
                return nc.vector.tensor_scalar(dst[:], src[:], 0.0, None, Alu.max)
            return nc.vector.tensor_scalar(
                dst[:], src[:], bias_col, 0.0, Alu.add, Alu.max
            )

        def evict_copy(dst, src, use_act=None):
            if use_act is None:
                use_act = (ecnt[0] * 5) % 9 < 5
            ecnt[0] += 1
            if use_act:
                return nc.scalar.activation(dst[:], src[:], Copy)
            return nc.vector.tensor_copy(dst[:], src[:])

        # Self-loading fp32r matmuls only have ONE sync-wait slot in walrus
        # codegen. Absorb extra cross-engine waits into PE nops placed just
        # before each matmul group: the nop waits, the PE FIFO order covers
        # the matmul, and Tile's vector clock elides the duplicate wait.
        def pe_absorb(producers):
            # returns nops (created BEFORE the matmuls they shield) — caller
            # must order the first matmul after them via pe_order()
            nops = []
            for p in producers:
                if p is None:
                    continue
                n = nc.tensor.nop()
                add_dep_helper(n.ins, p.ins, True, "wait-carrier")
                nops.append(n)
            return nops

        def pe_order(first_mm, nops):
            for n in nops:
                add_dep_helper(first_mm.ins, n.ins, False, "carrier-order")

        # slot-freeing instruction trackers (bufs=2 pools)
        zslot = deque(maxlen=2)   # tag "z1" (z1 tiles + op tiles)
        z2slot = deque(maxlen=2)  # z2 tiles

        wdmas = [wd_wpk, wd_w2, wd_b2, wd_w3]
        for bt in range(NBT):
            h2s = {}
            for g in range(NG):
                xo = xo_pool.tile([P, BT], f32r)
                sl = slice(bt * BT, (bt + 1) * BT)
                xdA = nc.sync.dma_start(xo[0:2, :], xa[4 * g : 4 * g + 2, sl])
                xdB = nc.sync.dma_start(xo[32:34, :], xa[4 * g + 2 : 4 * g + 4, sl])
                xo_deps = [xdA, xdB] + wdmas
                wdmas = []

                for wp in range(2):
                    unit_act = (2 * g + wp) % 2 == 0
                    h1s = []
                    for q in range(2):
                        w = 2 * wp + q
                        deps = list(xo_deps)
                        xo_deps = []
                        if len(zslot) == zslot.maxlen:
                            deps.append(zslot[0])
                        nops = pe_absorb(deps)
                        z1 = z1_pool.tile([P, 2 * W_], f32)
                        mm0 = None
                        for s in range(2):
                            mm = nc.tensor.matmul(
                                z1[:, s * W_ : (s + 1) * W_],
                                lhsT=wpk_sb[32 * s : 32 * s + 2, g * H : (g + 1) * H],
                                rhs=xo[32 * s : 32 * s + 2, w * W_ : (w + 1) * W_],
                                start=True,
                                stop=True,
                                tile_position=(32 * s, 0),
                            )
                            mm0 = mm0 or mm
                        pe_order(mm0, nops)
                        h1 = h1_pool.tile([P, 2 * W_], f32r)
                        zslot.append(evict(h1, z1, None, use_act=unit_act))
                        h1s.append(h1)
                    for s in range(2):
                        di = 2 * g + s
                        nops = (
                            pe_absorb([z2slot[0]])
                            if len(z2slot) == z2slot.maxlen
                            else []
                        )
                        z2 = z2_pool.tile([P, 2 * W_], f32)
                        mm0 = None
                        for q in range(2):
                            mm = nc.tensor.matmul(
                                z2[:, q * W_ : (q + 1) * W_],
                                lhsT=w2_sb[:, di * H : (di + 1) * H],
                                rhs=h1s[q][:, s * W_ : (s + 1) * W_],
                                start=True,
                                stop=True,
                            )
                            mm0 = mm0 or mm
                        pe_order(mm0, nops)
                        h2 = h2_pool.tile([P, 2 * W_], f32r)
                        z2slot.append(
                            evict(h2, z2, b2_sb[:, di : di + 1], use_act=unit_act)
                        )
                        h2s[(di, wp)] = h2

            # L3 for the whole bt chunk: for each 512-window accumulate all 8
            # components into PSUM partitions 0..7 (W3-embedding stationaries).
            for w in range(4):
                wp, q = w // 2, w % 2
                # share the z1 pool's PSUM slots — op is tiny and the L3
                # phase interleaves with the next bt's L1 fills
                nops = pe_absorb([zslot[0]]) if len(zslot) == zslot.maxlen else []
                op = z1_pool.tile([DPC, W_], f32, tag="z1")
                mm0 = None
                for di in range(DPC):
                    mm = nc.tensor.matmul(
                        op[:, :],
                        lhsT=w3_sb[:, DPC * di : DPC * (di + 1)],
                        rhs=h2s[(di, wp)][:, q * W_ : (q + 1) * W_],
                        start=(di == 0),
                        stop=(di == DPC - 1),
                    )
                    mm0 = mm0 or mm
                pe_order(mm0, nops)
                ost = ost_pool.tile([DPC, W_], f32)
                zslot.append(evict_copy(ost, op[:]))
                nc.sync.dma_start(o[bt, w], ost[:])

    return nc


def _build_program_raw():
    """Raw-bass build: manual engine streams + counting semaphores.

    Self-loading fp32r matmuls only support ONE sync-wait in walrus codegen,
    so all multi-proc waits are standalone wait_ge instructions placed by
    hand. PSUM: 4 ping-pong pairs of [128, 1024] (z1 A/B, z2 A/B); the L3
    `op` accumulators time-share the z1 banks at each bt boundary.
    """
    from concourse import bass, mybir

    f32 = mybir.dt.float32
    f32r = mybir.dt.float32r
    Relu = mybir.ActivationFunctionType.Relu
    Copy = mybir.ActivationFunctionType.Copy
    Alu = mybir.AluOpType

    nc = bass.Bass("TRN2", target_bir_lowering=False, debug=False)

    xa = nc.dram_tensor("xa", [2 * DPC, B], f32r, kind="ExternalInput")
    wpk = nc.dram_tensor("wpk", [P, NG * H], f32r, kind="ExternalInput")
    w2 = nc.dram_tensor("w2", [H, DPC * H], f32r, kind="ExternalInput")
    b2t = nc.dram_tensor("b2t", [H, DPC], f32, kind="ExternalInput")
    w3e = nc.dram_tensor("w3e", [H, DPC * DPC], f32r, kind="ExternalInput")
    o = nc.dram_tensor("o", [NBT, BT // W_, DPC, W_], f32, kind="ExternalOutput")

    # SBUF
    wpk_sb = nc.alloc_sbuf_tensor("wpk_sb", [P, NG * H], f32r)
    w2_sb = nc.alloc_sbuf_tensor("w2_sb", [H, DPC * H], f32r)
    b2_sb = nc.alloc_sbuf_tensor("b2_sb", [H, DPC], f32)
    w3_sb = nc.alloc_sbuf_tensor("w3_sb", [H, DPC * DPC], f32r)
    xo = [nc.alloc_sbuf_tensor(f"xo{i}", [P, BT], f32r) for i in range(2)]
    h1b = [nc.alloc_sbuf_tensor(f"h1b{i}", [P, 2 * W_], f32r) for i in range(4)]
    h2b = [
        [nc.alloc_sbuf_tensor(f"h2b{wp}_{d}", [P, 2 * W_], f32r) for d in range(DPC)]
        for wp in range(2)
    ]
    ost = [nc.alloc_sbuf_tensor(f"ost{i}", [DPC, W_], f32) for i in range(4)]

    # PSUM: zb0/zb1 = z1 ping-pong (+ L3 op at bt ends), zb2/zb3 = z2 ping-pong
    zb = [nc.alloc_psum_tensor(f"zb{i}", [P, 2 * W_], f32) for i in range(4)]

    # semaphores
    s_wdma = nc.alloc_semaphore("s_wdma")
    s_x = [nc.alloc_semaphore(f"s_x{i}") for i in range(2)]
    s_od = [nc.alloc_semaphore(f"s_od{i}") for i in range(4)]
    s_z1 = nc.alloc_semaphore("s_z1")
    s_z2 = nc.alloc_semaphore("s_z2")
    s_op = nc.alloc_semaphore("s_op")
    s_h1 = {"a": nc.alloc_semaphore("s_h1a"), "d": nc.alloc_semaphore("s_h1d")}
    s_h2 = {"a": nc.alloc_semaphore("s_h2a"), "d": nc.alloc_semaphore("s_h2d")}
    s_oc = {"a": nc.alloc_semaphore("s_oca"), "d": nc.alloc_semaphore("s_ocd")}

    NU = NBT * NG * 2  # 64 units; unit u = (bt, g, wp)

    # Chain-to-engine mapping: fill index j (j = 2u + q for z1, 2u + s for
    # z2) has parity-based ownership: even -> ACT ("a"), odd -> DVE ("d").
    # Each engine serves its chains strictly in order, so the engine's
    # counting semaphore value for evict j is simply j//2 + 1.
    def ev_eng(j):
        return "a" if j % 2 == 0 else "d"

    with nc.Block() as block:

        @block.sync
        def _(sp):
            sp.dma_start(wpk_sb[:, :], wpk[:, :]).then_inc(s_wdma, 16)
            sp.dma_start(w2_sb[:, :], w2[:, :]).then_inc(s_wdma, 16)
            sp.dma_start(b2_sb[:, :], b2t[:, :]).then_inc(s_wdma, 16)
            sp.dma_start(w3_sb[:, :], w3e[:, :]).then_inc(s_wdma, 16)
            for bt in range(NBT + 1):
                if bt < NBT:
                    for g in range(NG):
                        idx = bt * NG + g
                        xi = idx % 2
                        if idx >= 2:
                            # xo[xi] last read by L1 fills of (bt,g)-2:
                            # those are z1 fills 4*(idx-2)+1 .. 4*(idx-1)
                            sp.wait_ge(s_z1, 4 * (idx - 1))
                        sl = slice(bt * BT, (bt + 1) * BT)
                        sp.dma_start(
                            xo[xi][0:2, :], xa[4 * g : 4 * g + 2, sl]
                        ).then_inc(s_x[xi], 16)
                        sp.dma_start(
                            xo[xi][32:34, :], xa[4 * g + 2 : 4 * g + 4, sl]
                        ).then_inc(s_x[xi], 16)
                # out DMAs of the previous bt (out-copies all run on ACT)
                if bt >= 1:
                    for w in range(4):
                        k = (bt - 1) * 4 + w
                        sp.wait_ge(s_oc["a"], k + 1)
                        sp.dma_start(o[bt - 1, w], ost[k % 4][:, :]).then_inc(
                            s_od[k % 4], 16
                        )

        UPB = NG * 2  # units per bt

        ENG_OF = ("a", "d")

        def pe_z1_fill(pe, u, q):
            # one z1 fill (unit u, window-pair column q) into zb[q]
            bt, r = divmod(u, UPB)
            g, wp = r // 2, r % 2
            idx = bt * NG + g
            xi = idx % 2
            if wp == 0 and q == 0:
                pe.wait_ge(s_x[xi], 32 * (idx // 2 + 1))
            mm = None
            for s in range(2):
                mm = pe.matmul(
                    zb[q][:, s * W_ : (s + 1) * W_],
                    lhsT=wpk_sb[32 * s : 32 * s + 2, g * H : (g + 1) * H],
                    rhs=xo[xi][32 * s : 32 * s + 2, w_slice(wp, q)],
                    start=True,
                    stop=True,
                    tile_position=(32 * s, 0),
                )
                if u >= 1:
                    # WAR: the s-slice of fill 2(u-1)+q was evicted by
                    # engine s's half-evict
                    mm._wait_ge(s_h1[ENG_OF[s]], 2 * (u - 1) + q + 1)
            mm.then_inc(s_z1, 1)

        def pe_z2_fill(pe, v, s):
            bt, r = divmod(v, UPB)
            g, wp = r // 2, r % 2
            di = 2 * g + s
            mm = None
            for q in range(2):
                mm = pe.matmul(
                    zb[2 + s][:, q * W_ : (q + 1) * W_],
                    lhsT=w2_sb[:, di * H : (di + 1) * H],
                    rhs=h1b[(v % 2) * 2 + q][:, s * W_ : (s + 1) * W_],
                    start=True,
                    stop=True,
                )
                # ready: h1b fill 2v+q's s-half (engine s) evicted
                mm._wait_ge(s_h1[ENG_OF[s]], 2 * v + q + 1)
            mm.then_inc(s_z2, 1)

        def pe_l3_phase(pe, bt):
            # op(w) lives in zb[2 + w % 2][0:8, (w // 2)*512 :] — the z2
            # banks, so the next bt's z1 chains flow undisturbed.
            pe.wait_ge(s_h2["a"], 2 * UPB * (bt + 1))
            pe.wait_ge(s_h2["d"], 2 * UPB * (bt + 1))
            for w in range(4):
                wp, q = w // 2, w % 2
                opv = zb[2 + w % 2][0:DPC, (w // 2) * W_ : (w // 2 + 1) * W_]
                mm = None
                for di in range(DPC):
                    mm = pe.matmul(
                        opv,
                        lhsT=w3_sb[:, DPC * di : DPC * (di + 1)],
                        rhs=h2b[wp][di][:, q * W_ : (q + 1) * W_],
                        start=(di == 0),
                        stop=(di == DPC - 1),
                    )
                mm.then_inc(s_op, 1)

        @block.tensor
        def _(pe):
            pe.wait_ge(s_wdma, 64)
            for t in range(NU + 1):
                # slot t (spread order): zb0 fill early, z2 fills mid,
                # L3 phase at bt boundaries, zb1 fill late.
                if t < NU:
                    pe_z1_fill(pe, t, 0)
                if t >= 1:
                    v = t - 1
                    if v >= 1:
                        # zb2/zb3 WAR: both half-evicts of fills 2(v-1)+s
                        pe.wait_ge(s_h2["a"], 2 * v)
                        pe.wait_ge(s_h2["d"], 2 * v)
                    if v % UPB == 0 and v // UPB > 0:
                        # zb2/zb3 op regions read by out-copies of prev bt
                        pe.wait_ge(s_oc["a"], 4 * (v // UPB))
                    pe_z2_fill(pe, v, 0)
                    pe_z2_fill(pe, v, 1)
                if t < NU:
                    pe_z1_fill(pe, t, 1)
                if t >= 1 and t % UPB == 0:
                    # L3 after the trailing z1 fill so both evictors have
                    # h1 work queued while PE runs the 32 op matmuls
                    pe_l3_phase(pe, t // UPB - 1)

        # Each eviction is split in half along the free dim: ACT does
        # [:, 0:512], DVE does [:, 512:1024], concurrently. Engine sem
        # count for fill j is then j+1 on BOTH s_h1a/s_h1d (resp. h2).
        def ev_h1_half(eng, mine, u, q):
            par = 0 if mine == "a" else 1
            j = 2 * u + q
            hs = slice(par * W_, (par + 1) * W_)
            if u >= 2:
                # h1b[(u%2)*2+q] last read by L2 fills of unit u-2
                eng.wait_ge(s_z2, 2 * (u - 2) + 2)
            dst = h1b[(u % 2) * 2 + q][:, hs]
            ins = (
                eng.activation(dst, zb[q][:, hs], Relu)
                if mine == "a"
                else eng.tensor_scalar(dst, zb[q][:, hs], 0.0, None, Alu.max)
            )
            ins._wait_ge(s_z1, j + 1)
            ins.then_inc(s_h1[mine], 1)

        def ev_h2_half(eng, mine, v, s):
            par = 0 if mine == "a" else 1
            bt, r = divmod(v, NG * 2)
            g, wp = r // 2, r % 2
            j = 2 * v + s
            di = 2 * g + s
            hs = slice(par * W_, (par + 1) * W_)
            if bt > 0 and r == 0 and s == 0:
                eng.wait_ge(s_op, 4 * bt)  # h2b reuse WAR
            dst = h2b[wp][di][:, hs]
            ins = (
                eng.activation(dst, zb[2 + s][:, hs], Relu, bias=b2_sb[:, di : di + 1])
                if mine == "a"
                else eng.tensor_scalar(
                    dst,
                    zb[2 + s][:, hs],
                    b2_sb[:, di : di + 1],
                    0.0,
                    Alu.add,
                    Alu.max,
                )
            )
            ins._wait_ge(s_z2, j + 1)
            ins.then_inc(s_h2[mine], 1)

        def evict_stream(eng, mine):
            eng.wait_ge(s_wdma, 64)
            for t in range(NU + 1):
                if t < NU:
                    ev_h1_half(eng, mine, t, 0)
                if t >= 1:
                    ev_h2_half(eng, mine, t - 1, 0)
                    ev_h2_half(eng, mine, t - 1, 1)
                if t < NU:
                    ev_h1_half(eng, mine, t, 1)
                if t >= 1 and t % (NG * 2) == 0 and mine == "a":
                    bt = t // (NG * 2) - 1
                    for w in range(4):
                        k = bt * 4 + w
                        if k >= 4:
                            eng.wait_ge(s_od[k % 4], 16 * (k // 4))
                        opv = zb[2 + w % 2][
                            0:DPC, (w // 2) * W_ : (w // 2 + 1) * W_
                        ]
                        ins = eng.activation(ost[k % 4][:, :], opv, Copy)
                        ins._wait_ge(s_op, k + 1)
                        ins.then_inc(s_oc["a"], 1)

        @block.scalar
        def _(act):
            evict_stream(act, "a")

        @block.vector
        def _(dve):
            evict_stream(dve, "d")

    return nc


def w_slice(wp, q):
    w = 2 * wp + q
    return slice(w * W_, (w + 1) * W_)


def _prep_inputs(x, W1, b1, W2, b2, W3):
    """Build the per-core input maps (host-side shard + layout transforms)."""
    in_maps = []
    for c in range(NCORES):
        dlo = c * DPC
        dc = slice(dlo, dlo + DPC)

        xa = np.empty((2 * DPC, B), np.float32)
        xa[0::2] = x.T[dc]
        xa[1::2] = 1.0

        wpk = np.zeros((P, NG * H), np.float32)
        for g in range(NG):
            for s in range(2):
                d = dlo + 2 * g + s
                wpk[32 * s, g * H : (g + 1) * H] = W1[d]
                wpk[32 * s + 1, g * H : (g + 1) * H] = b1[d]

        w2c = np.ascontiguousarray(
            W2[dc].transpose(1, 0, 2).reshape(H, DPC * H)
        ).astype(np.float32)

        w3e = np.zeros((H, DPC * DPC), np.float32)
        for i in range(DPC):
            w3e[:, DPC * i + i] = W3[dlo + i]

        in_maps.append(
            {
                "xa": xa,
                "wpk": wpk,
                "w2": w2c,
                "b2t": np.ascontiguousarray(b2[dc].T).astype(np.float32),
                "w3e": w3e,
            }
        )
    return in_maps


def run_on_hw(in_maps, trace=False):
    from concourse.bass_utils import run_bass_kernel_spmd

    if "nc" not in _CACHE:
        _CACHE["nc"] = _build_program_raw()
    nc = _CACHE["nc"]
    res = run_bass_kernel_spmd(
        nc, in_maps, list(range(NCORES)), trace=trace
    )
    return res


def _gather(results, b3):
    out = np.empty((B, D), np.float32)
    for c in range(NCORES):
        dlo = c * DPC
        # o is [bt, w, d, 512] -> [d, B]
        oc = results[c]["o"].transpose(2, 0, 1, 3).reshape(DPC, B)
        out[:, dlo : dlo + DPC] = (oc + b3[dlo : dlo + DPC][:, None]).T
    return out


def kernel(x, W1, b1, W2, b2, W3, b3):
    x = np.asarray(x, np.float32)
    W1 = np.asarray(W1, np.float32)
    b1 = np.asarray(b1, np.float32)
    W2 = np.asarray(W2, np.float32)
    b2 = np.asarray(b2, np.float32)
    W3 = np.asarray(W3, np.float32)
    b3 = np.asarray(b3, np.float32)

    in_maps = _prep_inputs(x, W1, b1, W2, b2, W3)
    res = run_on_hw(in_maps)
    return _gather(res.results, b3)
